# revision 14
# baseline (speedup 1.0000x reference)
"""Trainium2 kernel for NeuralDictionaryV15 (retrieval_knn, top-1 softmax dictionary).

Reference computation:
    logits = keys @ query            # [N]
    att    = softmax(logits)         # [N]
    mask   = att >= max(att)         # top-1 (ties kept)
    out    = (mask * att) @ values   # [V]

Device work: the full N x D logits scan — the only part that must stream big
data. keys are quantized to fp8e4m3 on the host (4x less HBM traffic than f32;
DMA is the roofline for this memory-regime problem) and pre-arranged into the
exact SBUF tile layout the TensorEngine wants, so each DMA is one contiguous
blast. The dot products run on the PE array in DoubleRow fp8 mode (2 MACs per
cell per cycle: ~27us of PE time per core vs ~45us of DMA), contraction over
partitions, accumulated in f32 PSUM.

Sharding: keys row-sharded across 8 cores (32768 rows each). Each core emits
its 32768 f32 logits. The host then does what the reference does, exactly:
fp8-quantization error is bounded (|err| < ~5 on logits whose top-2 gap is
O(2)), so every row whose fp8 logit is within MARGIN of the fp8 max is
re-scored exactly in f32 on the host (~100-200 rows), giving the exact argmax
and an exact softmax numerator/denominator; rows outside the margin contribute
< 1e-10 to the denominator and are summed from their fp8 logits.
"""

import numpy as np
import ml_dtypes

N = 262144
D = 512
V = 512
NCORES = 8
NSHARD = N // NCORES          # 32768 rows per core
P = 128                       # SBUF partitions

# tunables
F = 4096                      # rows per tile (slab = [128, 2, F] fp8 = 1MB)
KBUFS = 8                     # key slab buffers
MARGIN = 35.0                 # fp8-logit margin for host exact rescore
ALT_DMA = False               # alternate sync/scalar HWDGE rings for slab DMAs

_CACHE = {}


def _split_waits(nc):
    """Work around walrus/concourse skew: this walrus build accepts at most
    one semaphore wait per instruction, but Tile emits several. Move extra
    waits onto same-engine nops inserted just before the instruction."""
    import concourse.mybir as mybir
    import bass_rust

    cnt = 0
    for f in nc.m.functions:
        for blk in f.blocks:
            newlist = []
            for ins in blk.instructions:
                si = ins.sync_info
                waits = list(si.on_wait) if si and si.on_wait else []
                if len(waits) > 1:
                    for w in waits[:-1]:
                        nop = bass_rust.InstNoOp(name=f"{ins.name}-wsplit{cnt}")
                        cnt += 1
                        nop.engine = ins.engine
                        nop.sync_info = mybir.SyncInfo(on_wait=[w], on_update=[])
                        newlist.append(nop)
                    ins.sync_info = mybir.SyncInfo(
                        on_wait=[waits[-1]],
                        on_update=list(si.on_update) if si.on_update else [],
                    )
                newlist.append(ins)
            blk.instructions = newlist
    return cnt


def _build_nc(f=None, kbufs=None, alt_dma=None, double_row=True):
    import concourse.bass as bass
    import concourse.mybir as mybir
    from concourse.tile import TileContext

    f = F if f is None else f
    kbufs = KBUFS if kbufs is None else kbufs
    alt_dma = ALT_DMA if alt_dma is None else alt_dma

    nt_total = NSHARD // f        # row tiles per core
    pieces = f // 512             # psum pieces per row tile (1 bank each;
                                  # DoubleRow matmul dst must be partition 0)

    nc = bass.Bass()
    # slab s = nt*2 + g, laid out [p, j, f]: value = keys8[nt*f + ff, g*256 + j*128 + p]
    kt = nc.declare_dram_parameter(
        "kt", [nt_total * 2, P, 2 * f], mybir.dt.float8e4, isOutput=False
    )
    # q8[p, j, c] = q8_flat[c*256 + j*128 + p] for c in {0,1}; c in 2..15 is
    # zero padding so the DoubleRow LDWEIGHTS pair-dim step is 16 (ISA req).
    q8 = nc.declare_dram_parameter("q8", [P, 2, 16], mybir.dt.float8e4, isOutput=False)
    logits = nc.declare_dram_parameter(
        "logits", [nt_total, f], mybir.dt.float32, isOutput=True
    )

    pm = mybir.MatmulPerfMode.DoubleRow if double_row else None

    with TileContext(nc) as tc:
        with (
            tc.tile_pool(name="ktiles", bufs=kbufs) as kpool,
            tc.tile_pool(name="psum", bufs=1, space="PSUM") as ppool,
            tc.tile_pool(name="stage", bufs=2) as spool,
            tc.tile_pool(name="singles", bufs=1) as singles,
        ):
            qt = singles.tile([P, 2, 16], mybir.dt.float8e4)
            nc.gpsimd.dma_start(out=qt[:], in_=q8[:])

            for nt in range(nt_total):
                slabs = []
                for g in range(2):
                    ktile = kpool.tile([P, 2, f], mybir.dt.float8e4)
                    eng = nc.scalar if (alt_dma and (nt * 2 + g) % 2) else nc.sync
                    eng.dma_start(out=ktile[:], in_=kt[nt * 2 + g])
                    slabs.append(ktile)
                stile = spool.tile([1, f], mybir.dt.float32)
                ptiles = [
                    ppool.tile(
                        [1, 512], mybir.dt.float32,
                        name=f"pt_{nt}_{i}", tag=f"pt{i}",
                    )
                    for i in range(pieces)
                ]
                # g-outer: one LDWEIGHTS per d-group, matmults reuse the
                # loaded stationary (ldweights=False) — saves ~100ns/matmul.
                for g in range(2):
                    if double_row:
                        nc.tensor.ldweights(qt[:, :, g : g + 1], perf_mode=pm)
                    for i in range(pieces):
                        sl = slice(i * 512, (i + 1) * 512)
                        if double_row:
                            mm = nc.tensor.matmul(
                                ptiles[i][:],
                                lhsT=qt[:, :, g : g + 1],
                                rhs=slabs[g][:, :, sl],
                                start=(g == 0),
                                stop=(g == 1),
                                perf_mode=pm,
                            )
                            mm.ldweights = False
                        else:
                            for j in range(2):
                                nc.tensor.matmul(
                                    ptiles[i][:],
                                    lhsT=qt[:, j : j + 1, g : g + 1],
                                    rhs=slabs[g][:, j, sl],
                                    start=(g == 0 and j == 0),
                                    stop=(g == 1 and j == 1),
                                )
                for i in range(pieces):
                    sl = slice(i * 512, (i + 1) * 512)
                    if i % 2:
                        nc.vector.tensor_copy(out=stile[:, sl], in_=ptiles[i][:])
                    else:
                        nc.scalar.copy(out=stile[:, sl], in_=ptiles[i][:])
                nc.sync.dma_start(out=logits[nt : nt + 1, :], in_=stile[:])
    _split_waits(nc)
    return nc


def _get_nc():
    if "nc" not in _CACHE:
        _CACHE["nc"] = _build_nc()
    return _CACHE["nc"]


def _pack_inputs(keys, query, f):
    """Quantize to fp8e4m3 and pre-arrange into the device tile layout."""
    nt_total = NSHARD // f
    k8 = np.clip(keys, -240.0, 240.0).astype(ml_dtypes.float8_e4m3)
    q8f = np.clip(query, -240.0, 240.0).astype(ml_dtypes.float8_e4m3)
    # kt[core, nt, g, p, j, ff] = k8[core*NSHARD + nt*f + ff, g*256 + j*128 + p]
    k8r = k8.reshape(NCORES, nt_total, f, 2, 2, P)      # [c, nt, ff, g, j, p]
    kt = np.ascontiguousarray(k8r.transpose(0, 1, 3, 5, 4, 2)).reshape(
        NCORES, nt_total * 2, P, 2 * f
    )
    # q8[p, j, c]: c 0/1 = d-group, 2..15 zero pad (DoubleRow pair step 16)
    qt = np.zeros((P, 2, 16), dtype=ml_dtypes.float8_e4m3)
    qt[:, :, :2] = q8f.reshape(2, 2, P).transpose(2, 1, 0)
    return kt, qt, k8, q8f


def _run_device(keys, query, trace=False, nc=None, f=None):
    """Run the per-core fp8 logits kernel on 8 cores; return [N] f32 fp8-logits."""
    from concourse.bass_utils import run_bass_kernel_spmd

    f = F if f is None else f
    if nc is None:
        nc = _get_nc()
    kt, qt, _, _ = _pack_inputs(keys, query, f)
    in_maps = [{"kt": kt[c], "q8": qt} for c in range(NCORES)]
    out = run_bass_kernel_spmd(nc, in_maps, core_ids=list(range(NCORES)), trace=trace)
    logits = np.concatenate([r["logits"].reshape(-1) for r in out.results])
    return logits, out


def _finish(logits8, query, keys, values, margin=None):
    """Exact host fixup: rescore every row whose fp8 logit is within `margin`
    of the fp8 max (fp8 logit error is < ~5; rows further out contribute
    < 1e-10 to the softmax denominator), then replicate the reference
    softmax/mask/matvec in f32 on the candidates."""
    margin = MARGIN if margin is None else margin
    q32 = query.astype(np.float32, copy=False)
    m8 = logits8.max()
    cand = np.nonzero(logits8 >= m8 - margin)[0]
    lc = keys[cand].astype(np.float32) @ q32          # exact f32 logits
    mc = lc.max()
    ec = np.exp(lc - mc, dtype=np.float32)
    tail_mask = np.ones(logits8.shape[0], bool)
    tail_mask[cand] = False
    tail = np.exp(logits8[tail_mask] - mc, dtype=np.float32).sum(dtype=np.float32)
    z = ec.sum(dtype=np.float32) + tail
    att = ec / z
    amax = att.max()
    sel = att >= amax
    rows = cand[sel]
    out = (att[sel][:, None] * values[rows].astype(np.float32)).sum(axis=0)
    return out.astype(np.float32)


def kernel(query, keys, values):
    query = np.asarray(query, dtype=np.float32)
    keys = np.asarray(keys, dtype=np.float32)
    values = np.asarray(values)
    logits8, _ = _run_device(keys, query, trace=False)
    return _finish(logits8, query, keys, values)


# revision 15
# speedup vs baseline: 1.1412x; 1.1412x over previous
"""Trainium2 kernel for NeuralDictionaryV15 (retrieval_knn, top-1 softmax dictionary).

Reference computation:
    logits = keys @ query            # [N]
    att    = softmax(logits)         # [N]
    mask   = att >= max(att)         # top-1 (ties kept)
    out    = (mask * att) @ values   # [V]

Device work: the full N x D logits scan — the only part that must stream big
data. keys are quantized to fp8e4m3 on the host (4x less HBM traffic than f32;
DMA is the roofline for this memory-regime problem) and pre-arranged into the
exact SBUF tile layout the TensorEngine wants, so each DMA is one contiguous
blast. The dot products run on the PE array in DoubleRow fp8 mode (2 MACs per
cell per cycle: ~27us of PE time per core vs ~45us of DMA), contraction over
partitions, accumulated in f32 PSUM.

Sharding: keys row-sharded across 8 cores (32768 rows each). Each core emits
its 32768 f32 logits. The host then does what the reference does, exactly:
fp8-quantization error is bounded (|err| < ~5 on logits whose top-2 gap is
O(2)), so every row whose fp8 logit is within MARGIN of the fp8 max is
re-scored exactly in f32 on the host (~100-200 rows), giving the exact argmax
and an exact softmax numerator/denominator; rows outside the margin contribute
< 1e-10 to the denominator and are summed from their fp8 logits.
"""

import numpy as np
import ml_dtypes

N = 262144
D = 512
V = 512
NCORES = 8
NSHARD = N // NCORES          # 32768 rows per core
P = 128                       # SBUF partitions

# tunables
F = 4096                      # rows per tile (slab = [128, 2, F] fp8 = 1MB)
KBUFS = 8                     # key slab buffers
MARGIN = 35.0                 # fp8-logit margin for host exact rescore
ALT_DMA = False               # alternate sync/scalar HWDGE rings for slab DMAs

_CACHE = {}


def _split_waits(nc):
    """Work around walrus/concourse skew: this walrus build accepts at most
    one semaphore wait per instruction, but Tile emits several. Move extra
    waits onto same-engine nops inserted just before the instruction."""
    import concourse.mybir as mybir
    import bass_rust

    cnt = 0
    for f in nc.m.functions:
        for blk in f.blocks:
            newlist = []
            for ins in blk.instructions:
                si = ins.sync_info
                waits = list(si.on_wait) if si and si.on_wait else []
                if len(waits) > 1:
                    for w in waits[:-1]:
                        nop = bass_rust.InstNoOp(name=f"{ins.name}-wsplit{cnt}")
                        cnt += 1
                        nop.engine = ins.engine
                        nop.sync_info = mybir.SyncInfo(on_wait=[w], on_update=[])
                        newlist.append(nop)
                    ins.sync_info = mybir.SyncInfo(
                        on_wait=[waits[-1]],
                        on_update=list(si.on_update) if si.on_update else [],
                    )
                newlist.append(ins)
            blk.instructions = newlist
    return cnt


def _build_nc(f=None, kbufs=None, alt_dma=None, double_row=True):
    import concourse.bass as bass
    import concourse.mybir as mybir
    from concourse.tile import TileContext

    f = F if f is None else f
    kbufs = KBUFS if kbufs is None else kbufs
    alt_dma = ALT_DMA if alt_dma is None else alt_dma

    nt_total = NSHARD // f        # row tiles per core
    pieces = f // 512             # psum pieces per row tile (1 bank each;
                                  # DoubleRow matmul dst must be partition 0)

    nc = bass.Bass()
    # slab s = nt*2 + g, laid out [p, j, f]: value = keys8[nt*f + ff, g*256 + j*128 + p]
    kt = nc.declare_dram_parameter(
        "kt", [nt_total * 2, P, 2 * f], mybir.dt.float8e4, isOutput=False
    )
    # q8[p, j, c] = q8_flat[c*256 + j*128 + p] for c in {0,1}; c in 2..15 is
    # zero padding so the DoubleRow LDWEIGHTS pair-dim step is 16 (ISA req).
    q8 = nc.declare_dram_parameter("q8", [P, 2, 16], mybir.dt.float8e4, isOutput=False)
    logits = nc.declare_dram_parameter(
        "logits", [nt_total, f], mybir.dt.float32, isOutput=True
    )

    pm = mybir.MatmulPerfMode.DoubleRow if double_row else None

    with TileContext(nc) as tc:
        with (
            tc.tile_pool(name="ktiles", bufs=kbufs) as kpool,
            tc.tile_pool(name="psum", bufs=1, space="PSUM") as ppool,
            tc.tile_pool(name="stage", bufs=2) as spool,
            tc.tile_pool(name="singles", bufs=1) as singles,
        ):
            qt = singles.tile([P, 2, 16], mybir.dt.float8e4)
            nc.gpsimd.dma_start(out=qt[:], in_=q8[:])

            for nt in range(nt_total):
                slabs = []
                for g in range(2):
                    ktile = kpool.tile([P, 2, f], mybir.dt.float8e4)
                    eng = nc.scalar if (alt_dma and (nt * 2 + g) % 2) else nc.sync
                    eng.dma_start(out=ktile[:], in_=kt[nt * 2 + g])
                    slabs.append(ktile)
                stile = spool.tile([1, f], mybir.dt.float32)
                ptiles = [
                    ppool.tile(
                        [1, 512], mybir.dt.float32,
                        name=f"pt_{nt}_{i}", tag=f"pt{i}",
                    )
                    for i in range(pieces)
                ]
                # g-outer: one LDWEIGHTS per d-group, matmults reuse the
                # loaded stationary (ldweights=False) — saves ~100ns/matmul.
                for g in range(2):
                    if double_row:
                        nc.tensor.ldweights(qt[:, :, g : g + 1], perf_mode=pm)
                    for i in range(pieces):
                        sl = slice(i * 512, (i + 1) * 512)
                        if double_row:
                            mm = nc.tensor.matmul(
                                ptiles[i][:],
                                lhsT=qt[:, :, g : g + 1],
                                rhs=slabs[g][:, :, sl],
                                start=(g == 0),
                                stop=(g == 1),
                                perf_mode=pm,
                            )
                            mm.ins.ldweights = False
                        else:
                            for j in range(2):
                                nc.tensor.matmul(
                                    ptiles[i][:],
                                    lhsT=qt[:, j : j + 1, g : g + 1],
                                    rhs=slabs[g][:, j, sl],
                                    start=(g == 0 and j == 0),
                                    stop=(g == 1 and j == 1),
                                )
                for i in range(pieces):
                    sl = slice(i * 512, (i + 1) * 512)
                    if i % 2:
                        nc.vector.tensor_copy(out=stile[:, sl], in_=ptiles[i][:])
                    else:
                        nc.scalar.copy(out=stile[:, sl], in_=ptiles[i][:])
                nc.sync.dma_start(out=logits[nt : nt + 1, :], in_=stile[:])
    _split_waits(nc)
    return nc


def _get_nc():
    if "nc" not in _CACHE:
        _CACHE["nc"] = _build_nc()
    return _CACHE["nc"]


def _pack_inputs(keys, query, f):
    """Quantize to fp8e4m3 and pre-arrange into the device tile layout."""
    nt_total = NSHARD // f
    k8 = np.clip(keys, -240.0, 240.0).astype(ml_dtypes.float8_e4m3)
    q8f = np.clip(query, -240.0, 240.0).astype(ml_dtypes.float8_e4m3)
    # kt[core, nt, g, p, j, ff] = k8[core*NSHARD + nt*f + ff, g*256 + j*128 + p]
    k8r = k8.reshape(NCORES, nt_total, f, 2, 2, P)      # [c, nt, ff, g, j, p]
    kt = np.ascontiguousarray(k8r.transpose(0, 1, 3, 5, 4, 2)).reshape(
        NCORES, nt_total * 2, P, 2 * f
    )
    # q8[p, j, c]: c 0/1 = d-group, 2..15 zero pad (DoubleRow pair step 16)
    qt = np.zeros((P, 2, 16), dtype=ml_dtypes.float8_e4m3)
    qt[:, :, :2] = q8f.reshape(2, 2, P).transpose(2, 1, 0)
    return kt, qt, k8, q8f


def _run_device(keys, query, trace=False, nc=None, f=None):
    """Run the per-core fp8 logits kernel on 8 cores; return [N] f32 fp8-logits."""
    from concourse.bass_utils import run_bass_kernel_spmd

    f = F if f is None else f
    if nc is None:
        nc = _get_nc()
    kt, qt, _, _ = _pack_inputs(keys, query, f)
    in_maps = [{"kt": kt[c], "q8": qt} for c in range(NCORES)]
    out = run_bass_kernel_spmd(nc, in_maps, core_ids=list(range(NCORES)), trace=trace)
    logits = np.concatenate([r["logits"].reshape(-1) for r in out.results])
    return logits, out


def _finish(logits8, query, keys, values, margin=None):
    """Exact host fixup: rescore every row whose fp8 logit is within `margin`
    of the fp8 max (fp8 logit error is < ~5; rows further out contribute
    < 1e-10 to the softmax denominator), then replicate the reference
    softmax/mask/matvec in f32 on the candidates."""
    margin = MARGIN if margin is None else margin
    q32 = query.astype(np.float32, copy=False)
    m8 = logits8.max()
    cand = np.nonzero(logits8 >= m8 - margin)[0]
    lc = keys[cand].astype(np.float32) @ q32          # exact f32 logits
    mc = lc.max()
    ec = np.exp(lc - mc, dtype=np.float32)
    tail_mask = np.ones(logits8.shape[0], bool)
    tail_mask[cand] = False
    tail = np.exp(logits8[tail_mask] - mc, dtype=np.float32).sum(dtype=np.float32)
    z = ec.sum(dtype=np.float32) + tail
    att = ec / z
    amax = att.max()
    sel = att >= amax
    rows = cand[sel]
    out = (att[sel][:, None] * values[rows].astype(np.float32)).sum(axis=0)
    return out.astype(np.float32)


def kernel(query, keys, values):
    query = np.asarray(query, dtype=np.float32)
    keys = np.asarray(keys, dtype=np.float32)
    values = np.asarray(values)
    logits8, _ = _run_device(keys, query, trace=False)
    return _finish(logits8, query, keys, values)
